# revision 17
# baseline (speedup 1.0000x reference)
"""Trainium2 Bass kernel for MeanTokenProjectionPool.

Computes, for batch [B,T,D], per-type segmented masked mean over T into G
groups followed by a per-group linear projection (W[g] @ mean + b[g]).

Strategy (data-parallel over B, 4 batch items per core, no cross-core comm):
  - ~50% of tokens are padding (key_padding_mask) and contribute nothing.
    The host packs only the VALID tokens of each core's 4 batch items into
    a contiguous stream laid out exactly as SBUF wants it
    [128 part, chunk, 512], so the batch DMA is a handful of large fully
    contiguous transfers at near-peak HBM bandwidth.
  - rel-err budget is 2e-2; the batch streams as fp8 e3m4 (1 B/elem,
    pre-scaled by 2 to use the dynamic range; measured end-to-end rel err
    1.27e-2 on the real data). The 0/1 membership mask vf is e3m4 too
    (exact). W keeps fp16 (its values sit in e3m4's subnormal range, and
    the kernel is PE-column-bound, not byte-bound, by this point).
  - One PE matmul per 128-token chunk accumulates all 32 (b,g) sums into a
    single PSUM bank; a per-partition DVE multiply by invc[b,g]/2 then
    yields means[32, 512] in f32. Batch tiles ramp [8,16,...] chunks so
    the first matmul fires early, and warm-up junk matmuls hold the PE
    clock at 2.4 GHz through the DMA lead-in.
  - Phase 2: PE-transpose means into [128d, 4c x 32bg], cast fp16, then per
    group g: out_g[4b, 512o] += mT[:,c,{b*8+g}] @ W16[g,c] over 4 d-chunks,
    one PSUM bank per group. W streams as 8 per-group fp16 tiles after the
    batch on the same HWDGE FIFO, so each group's GEMM fires as its W
    slice lands; a per-group bias add moves PSUM->SBUF.
  - Output per core is [4, G*OUT] f32; host reshapes/concats over cores.
"""

import ml_dtypes
import numpy as np

import concourse.bacc as bacc
import concourse.mybir as mybir
from concourse import bass_utils

from concourse.tile import TileContext

B, T, D, G, OUT = 32, 4096, 512, 8, 512
NCORES = 8
BL = B // NCORES   # batch items per core (4)
P = 128
DCH = D // P       # contraction chunks for the projection (4)
BG = BL * G        # combined (b,g) segments per core (32)
BSCALE = 2.0       # batch pre-scale into e3m4 range; undone via invc
NWARM = 30         # junk matmuls that hold the PE clock up during DMA lead-in

F32 = mybir.dt.float32
F16 = mybir.dt.float16
BF16 = mybir.dt.bfloat16
F8E3 = mybir.dt.float8e3
NPF16 = np.float16
NPE3 = ml_dtypes.float8_e3m4

_cache: dict = {}


def _tile_sizes(ncp: int):
    """Tile sizes in chunks: one small lead tile so phase 1 starts early,
    16-chunk steady state, small remainder tiles last so the batch tail's
    completion semaphore does not strand the final matmuls."""
    sizes = []
    for s in (2, 4, 8):
        if sum(sizes) + s >= ncp:
            break
        sizes.append(s)
    while ncp - sum(sizes) > 16 + 8:
        sizes.append(16)
    rem = ncp - sum(sizes)
    sizes.append(rem - rem // 2)
    sizes.append(rem // 2)
    return [s for s in sizes if s > 0]


def _build(ncp: int):
    """Compile the SPMD program for a packed capacity of ncp 128-token
    chunks per core."""
    nc = bacc.Bacc(
        "TRN2", target_bir_lowering=False, debug=False, num_devices=NCORES
    )

    sizes = _tile_sizes(ncp)

    bt_d = nc.dram_tensor(
        "batch_pk", [P, ncp * (D + BG)], F8E3, kind="ExternalInput"
    )
    w_d = nc.dram_tensor("w16", [P, G * DCH * OUT], F16, kind="ExternalInput")
    bias_d = nc.dram_tensor("biasr", [BL, G * OUT], F32, kind="ExternalInput")
    invc_d = nc.dram_tensor("invc", [BG, 1], F32, kind="ExternalInput")
    ident_d = nc.dram_tensor("ident", [BG, BG], F32, kind="ExternalInput")
    out_d = nc.dram_tensor("out", [BL, G * OUT], F32, kind="ExternalOutput")

    with TileContext(nc) as tc:
        with tc.tile_pool(name="consts", bufs=1) as consts, \
             tc.tile_pool(name="bpool", bufs=len(sizes)) as bpool, \
             tc.tile_pool(name="wpool", bufs=G) as wpool:
            pacc_ctx = tc.tile_pool(name="pacc", bufs=1, space="PSUM")
            pacc = pacc_ctx.__enter__()
            ptp_ctx = tc.tile_pool(name="ptp", bufs=1, space="PSUM")
            ptp = ptp_ctx.__enter__()
            pjunk_ctx = tc.tile_pool(name="pjunk", bufs=1, space="PSUM")
            pjunk = pjunk_ctx.__enter__()

            # Entire input stream issues up-front on the sync HWDGE FIFO in
            # priority order; every tile has its own buffer so the ring
            # never stalls on buffer reuse. Small consts ride the scalar
            # HWDGE ring so they don't delay the batch stream. Each tile
            # carries its vf mask columns inline (host-interleaved), so
            # every phase-1 matmul waits on exactly ONE transfer's
            # completion semaphore.
            CW = D + BG
            btiles = []
            c0 = 0
            for s in sizes:
                bt = bpool.tile([P, s * CW], F8E3, tag="bt")
                nc.sync.dma_start(
                    out=bt, in_=bt_d.ap()[:, c0 * CW:(c0 + s) * CW]
                )
                btiles.append((c0, c0 + s, bt))
                c0 += s

            wtiles = []
            for g in range(G):
                wg = wpool.tile([P, DCH * OUT], F16, tag="wg")
                nc.sync.dma_start(
                    out=wg, in_=w_d.ap()[:, g * DCH * OUT:(g + 1) * DCH * OUT]
                )
                wtiles.append(wg)

            bias_sb = consts.tile([BL, G * OUT], F32)
            nc.scalar.dma_start(out=bias_sb, in_=bias_d.ap())
            invc_sb = consts.tile([BG, 1], F32)
            nc.scalar.dma_start(out=invc_sb, in_=invc_d.ap())
            ident = consts.tile([BG, BG], F32)
            nc.scalar.dma_start(out=ident, in_=ident_d.ap())

            means_sb = consts.tile([BG, D], F32)
            mt_sb = consts.tile([P, DCH, BG], F16)
            out_sb = consts.tile([BL, G * OUT], F32)

            # Junk matmuls: no data deps, so they run while the first DMAs
            # stream in, pushing the PE through the HAM half-clock window.
            junk_sb = consts.tile([P, P], BF16)
            nc.gpsimd.memset(junk_sb, 0.0)
            junk_ps = pjunk.tile([G, P], F32, tag="junk")
            for _ in range(NWARM):
                nc.tensor.matmul(
                    junk_ps, lhsT=junk_sb[:, :G], rhs=junk_sb,
                    start=True, stop=True,
                )

            # Phase 1: sums[32 (b,g), 512] accumulate in one PSUM bank.
            means_ps = pacc.tile([BG, D], F32, tag="means")
            for c0, c1, bt in btiles:
                s = c1 - c0
                for c in range(c0, c1):
                    nc.tensor.matmul(
                        means_ps,
                        lhsT=bt[:, s * D + (c - c0) * BG:
                                s * D + (c - c0 + 1) * BG],
                        rhs=bt[:, (c - c0) * D:(c - c0 + 1) * D],
                        start=(c == 0), stop=(c == ncp - 1),
                    )
            # means = sums * (invc / BSCALE), per-(b,g)-partition scalar
            nc.vector.tensor_scalar_mul(means_sb, means_ps, invc_sb)

            # Transpose means -> mt [128 d, c, (b,g)] and cast to fp16.
            tp_ps = ptp.tile([P, DCH, BG], F32, tag="tp")
            for c in range(DCH):
                nc.tensor.transpose(
                    tp_ps[:, c, :], means_sb[:, c * P:(c + 1) * P], ident
                )
            nc.vector.tensor_copy(out=mt_sb, in_=tp_ps)

            # Phase 2: per-group projection, one PSUM bank per group.
            pjunk_ctx.__exit__(None, None, None)
            ptp_ctx.__exit__(None, None, None)
            pacc_ctx.__exit__(None, None, None)
            mt_v = mt_sb.rearrange("p c (b g) -> p c g b", g=G)
            with tc.tile_pool(name="pout", bufs=G, space="PSUM") as pout:
                for g in range(G):
                    og = pout.tile([BL, OUT], F32, tag="og", name=f"og{g}")
                    for c in range(DCH):
                        nc.tensor.matmul(
                            og,
                            lhsT=mt_v[:, c, g, :],
                            rhs=wtiles[g][:, c * OUT:(c + 1) * OUT],
                            start=(c == 0), stop=(c == DCH - 1),
                        )
                    # bias add + PSUM->SBUF copyback in one op
                    nc.vector.tensor_add(
                        out_sb[:, g * OUT:(g + 1) * OUT],
                        og,
                        bias_sb[:, g * OUT:(g + 1) * OUT],
                    )

            nc.sync.dma_start(out=out_d.ap(), in_=out_sb)

    nc.compile()
    return nc


def _prep(inputs):
    batch = np.asarray(inputs["batch"], dtype=np.float32)
    W = np.asarray(inputs["W"], dtype=np.float32)
    b_bias = np.asarray(inputs["b_bias"], dtype=np.float32)
    tt = np.asarray(inputs["token_types"]).astype(np.int64)
    pad = np.asarray(inputs["key_padding_mask"]).astype(bool)

    valid = ~pad                                   # [B, T]
    onehot = tt[:, None] == np.arange(G)[None, :]  # [T, G]
    counts = valid.astype(np.float32) @ onehot.astype(np.float32)  # [B, G]
    invc = np.where(counts > 0, 1.0 / np.maximum(counts, 1.0), 0.0).astype(
        np.float32
    ) / BSCALE

    core_tok = valid.reshape(NCORES, BL * T).sum(axis=1)
    ncp = int(max(core_tok + P - 1) // P)

    # w16[p, (g*DCH + c)*OUT + o] = W[g, c*128 + p, o]
    w16 = np.ascontiguousarray(
        W.reshape(G, DCH, P, OUT).transpose(2, 0, 1, 3)
    ).reshape(P, G * DCH * OUT).astype(NPF16)
    biasr = np.ascontiguousarray(
        np.broadcast_to(b_bias.reshape(1, G * OUT), (BL, G * OUT))
    )
    ident = np.eye(BG, dtype=np.float32)

    in_maps = []
    for cidx in range(NCORES):
        bs = slice(BL * cidx, BL * (cidx + 1))
        vb = valid[bs]                      # [BL, T]
        ib, it = np.nonzero(vb)             # b-major, t ascending
        n = len(ib)

        pk = np.zeros((ncp * P, D), dtype=NPE3)
        pk[:n] = (batch[bs][ib, it] * BSCALE).astype(NPE3)
        pk = pk.reshape(ncp, P, D)

        vf = np.zeros((ncp * P, BG), dtype=NPE3)
        g_of = tt[it]
        vf[np.arange(n), ib * G + g_of] = np.float32(1.0)
        vf = vf.reshape(ncp, P, BG)

        blks = []
        c0 = 0
        for s in _tile_sizes(ncp):
            blks.append(
                pk[c0:c0 + s].transpose(1, 0, 2).reshape(P, s * D)
            )
            blks.append(
                vf[c0:c0 + s].transpose(1, 0, 2).reshape(P, s * BG)
            )
            c0 += s
        bt_dram = np.ascontiguousarray(np.concatenate(blks, axis=1))

        in_maps.append(
            {
                "batch_pk": bt_dram,
                "w16": w16,
                "biasr": biasr,
                "invc": np.ascontiguousarray(
                    invc[bs].reshape(BG, 1)
                ),
                "ident": ident,
            }
        )
    return ncp, in_maps


def _gather(results):
    outs = [np.asarray(r["out"]).reshape(BL, G, OUT) for r in results]
    return np.ascontiguousarray(np.concatenate(outs, axis=0))


def kernel(**inputs) -> np.ndarray:
    ncp, in_maps = _prep(inputs)
    key = ("nc", ncp)
    if key not in _cache:
        _cache[key] = _build(ncp)
    res = bass_utils.run_bass_kernel_spmd(
        _cache[key], in_maps, core_ids=list(range(NCORES))
    )
    return _gather(res.results)


# revision 18
# speedup vs baseline: 1.1151x; 1.1151x over previous
"""Trainium2 Bass kernel for MeanTokenProjectionPool.

Computes, for batch [B,T,D], per-type segmented masked mean over T into G
groups followed by a per-group linear projection (W[g] @ mean + b[g]).

Strategy (data-parallel over B, 4 batch items per core, no cross-core comm):
  - ~50% of tokens are padding (key_padding_mask) and contribute nothing.
    The host packs only the VALID tokens of each core's 4 batch items into
    a contiguous stream laid out exactly as SBUF wants it
    [128 part, chunk, 512], so the batch DMA is a handful of large fully
    contiguous transfers at near-peak HBM bandwidth.
  - rel-err budget is 2e-2; the batch streams as fp8 e3m4 (1 B/elem,
    pre-scaled by 2 to use the dynamic range; measured end-to-end rel err
    1.27e-2 on the real data). The 0/1 membership mask vf is e3m4 too
    (exact). W keeps fp16 (its values sit in e3m4's subnormal range, and
    the kernel is PE-column-bound, not byte-bound, by this point).
  - One PE matmul per 128-token chunk accumulates all 32 (b,g) sums into a
    single PSUM bank; a per-partition DVE multiply by invc[b,g]/2 then
    yields means[32, 512] in f32. Batch tiles ramp [8,16,...] chunks so
    the first matmul fires early, and warm-up junk matmuls hold the PE
    clock at 2.4 GHz through the DMA lead-in.
  - Phase 2: PE-transpose means into [128d, 4c x 32bg], cast fp16, then per
    group g: out_g[4b, 512o] += mT[:,c,{b*8+g}] @ W16[g,c] over 4 d-chunks,
    one PSUM bank per group. W streams as 8 per-group fp16 tiles after the
    batch on the same HWDGE FIFO, so each group's GEMM fires as its W
    slice lands; a per-group bias add moves PSUM->SBUF.
  - Output per core is [4, G*OUT] f32; host reshapes/concats over cores.
"""

import ml_dtypes
import numpy as np

import concourse.bacc as bacc
import concourse.mybir as mybir
from concourse import bass_utils

from concourse.tile import TileContext

B, T, D, G, OUT = 32, 4096, 512, 8, 512
NCORES = 8
BL = B // NCORES   # batch items per core (4)
P = 128
DCH = D // P       # contraction chunks for the projection (4)
BG = BL * G        # combined (b,g) segments per core (32)
BSCALE = 2.0       # batch pre-scale into e3m4 range; undone via invc
NWARM = 48         # junk matmuls that hold the PE clock up during DMA lead-in

F32 = mybir.dt.float32
F16 = mybir.dt.float16
BF16 = mybir.dt.bfloat16
F8E3 = mybir.dt.float8e3
NPF16 = np.float16
NPE3 = ml_dtypes.float8_e3m4

_cache: dict = {}


def _tile_sizes(ncp: int):
    """Tile sizes in chunks: one small lead tile so phase 1 starts early,
    16-chunk steady state, small remainder tiles last so the batch tail's
    completion semaphore does not strand the final matmuls."""
    sizes = []
    for s in (8,):
        if sum(sizes) + s >= ncp:
            break
        sizes.append(s)
    while ncp - sum(sizes) > 16 + 8:
        sizes.append(16)
    rem = ncp - sum(sizes)
    sizes.append(rem - rem // 2)
    sizes.append(rem // 2)
    return [s for s in sizes if s > 0]


def _build(ncp: int):
    """Compile the SPMD program for a packed capacity of ncp 128-token
    chunks per core."""
    nc = bacc.Bacc(
        "TRN2", target_bir_lowering=False, debug=False, num_devices=NCORES
    )

    sizes = _tile_sizes(ncp)

    bt_d = nc.dram_tensor("batch_pk", [P, ncp * D], F8E3, kind="ExternalInput")
    vf_d = nc.dram_tensor("vf", [P, ncp * BG], F8E3, kind="ExternalInput")
    w_d = nc.dram_tensor("w16", [P, G * DCH * OUT], F16, kind="ExternalInput")
    bias_d = nc.dram_tensor("biasr", [BL, G * OUT], F32, kind="ExternalInput")
    invc_d = nc.dram_tensor("invc", [BG, 1], F32, kind="ExternalInput")
    ident_d = nc.dram_tensor("ident", [BG, BG], F32, kind="ExternalInput")
    out_d = nc.dram_tensor("out", [BL, G * OUT], F32, kind="ExternalOutput")

    with TileContext(nc) as tc:
        with tc.tile_pool(name="consts", bufs=1) as consts, \
             tc.tile_pool(name="bpool", bufs=len(sizes)) as bpool, \
             tc.tile_pool(name="wpool", bufs=G) as wpool:
            pacc_ctx = tc.tile_pool(name="pacc", bufs=1, space="PSUM")
            pacc = pacc_ctx.__enter__()
            ptp_ctx = tc.tile_pool(name="ptp", bufs=1, space="PSUM")
            ptp = ptp_ctx.__enter__()
            pjunk_ctx = tc.tile_pool(name="pjunk", bufs=1, space="PSUM")
            pjunk = pjunk_ctx.__enter__()

            # Entire input stream issues up-front on the sync HWDGE FIFO in
            # priority order; every tile has its own buffer so the ring
            # never stalls on buffer reuse. Small consts ride the scalar
            # HWDGE ring so they don't delay the batch stream. vf splits
            # into a head (covering the lead tiles) and rest so the first
            # matmuls only wait on a tiny mask DMA.
            vfsplit = min(sum(sizes[:2]), ncp)
            vfh_sb = consts.tile([P, vfsplit * BG], F8E3)
            vfr_sb = consts.tile([P, (ncp - vfsplit) * BG], F8E3)

            def vf_col(c):
                if c < vfsplit:
                    return vfh_sb[:, c * BG:(c + 1) * BG]
                cc = c - vfsplit
                return vfr_sb[:, cc * BG:(cc + 1) * BG]

            btiles = []
            c0 = 0
            for i, s in enumerate(sizes):
                bt = bpool.tile([P, s * D], F8E3, tag="bt")
                btiles.append((c0, c0 + s, bt))
                c0 += s

            def issue_bt(i):
                c0, c1, bt = btiles[i]
                nc.sync.dma_start(
                    out=bt, in_=bt_d.ap()[:, c0 * D:c1 * D]
                )

            issue_bt(0)
            nc.sync.dma_start(out=vfh_sb, in_=vf_d.ap()[:, :vfsplit * BG])
            if len(sizes) > 1:
                issue_bt(1)
            nc.sync.dma_start(out=vfr_sb, in_=vf_d.ap()[:, vfsplit * BG:])
            for i in range(2, len(sizes)):
                issue_bt(i)

            wtiles = []
            for g in range(G):
                wg = wpool.tile([P, DCH * OUT], F16, tag="wg")
                nc.sync.dma_start(
                    out=wg, in_=w_d.ap()[:, g * DCH * OUT:(g + 1) * DCH * OUT]
                )
                wtiles.append(wg)

            bias_sb = consts.tile([BL, G * OUT], F32)
            nc.scalar.dma_start(out=bias_sb, in_=bias_d.ap())
            invc_sb = consts.tile([BG, 1], F32)
            nc.scalar.dma_start(out=invc_sb, in_=invc_d.ap())
            ident = consts.tile([BG, BG], F32)
            nc.scalar.dma_start(out=ident, in_=ident_d.ap())

            means_sb = consts.tile([BG, D], F32)
            mt_sb = consts.tile([P, DCH, BG], F16)
            out_sb = consts.tile([BL, G * OUT], F32)

            # Junk matmuls: no data deps, so they run while the first DMAs
            # stream in, pushing the PE through the HAM half-clock window.
            junk_sb = consts.tile([P, P], BF16)
            nc.gpsimd.memset(junk_sb, 0.0)
            junk_ps = pjunk.tile([G, P], F32, tag="junk")
            for _ in range(NWARM):
                nc.tensor.matmul(
                    junk_ps, lhsT=junk_sb[:, :G], rhs=junk_sb,
                    start=True, stop=True,
                )

            # Phase 1: sums[32 (b,g), 512] accumulate in one PSUM bank.
            means_ps = pacc.tile([BG, D], F32, tag="means")
            for c0, c1, bt in btiles:
                for c in range(c0, c1):
                    nc.tensor.matmul(
                        means_ps,
                        lhsT=vf_col(c),
                        rhs=bt[:, (c - c0) * D:(c - c0 + 1) * D],
                        start=(c == 0), stop=(c == ncp - 1),
                    )
            # means = sums * (invc / BSCALE), per-(b,g)-partition scalar
            nc.vector.tensor_scalar_mul(means_sb, means_ps, invc_sb)

            # Transpose means -> mt [128 d, c, (b,g)] and cast to fp16.
            tp_ps = ptp.tile([P, DCH, BG], F32, tag="tp")
            for c in range(DCH):
                nc.tensor.transpose(
                    tp_ps[:, c, :], means_sb[:, c * P:(c + 1) * P], ident
                )
            nc.vector.tensor_copy(out=mt_sb, in_=tp_ps)

            # Phase 2: per-group projection, one PSUM bank per group.
            pjunk_ctx.__exit__(None, None, None)
            ptp_ctx.__exit__(None, None, None)
            pacc_ctx.__exit__(None, None, None)
            mt_v = mt_sb.rearrange("p c (b g) -> p c g b", g=G)
            with tc.tile_pool(name="pout", bufs=G, space="PSUM") as pout:
                for g in range(G):
                    og = pout.tile([BL, OUT], F32, tag="og", name=f"og{g}")
                    for c in range(DCH):
                        nc.tensor.matmul(
                            og,
                            lhsT=mt_v[:, c, g, :],
                            rhs=wtiles[g][:, c * OUT:(c + 1) * OUT],
                            start=(c == 0), stop=(c == DCH - 1),
                        )
                    # bias add + PSUM->SBUF copyback in one op
                    nc.vector.tensor_add(
                        out_sb[:, g * OUT:(g + 1) * OUT],
                        og,
                        bias_sb[:, g * OUT:(g + 1) * OUT],
                    )

            nc.sync.dma_start(out=out_d.ap(), in_=out_sb)

    nc.compile()
    return nc


def _prep(inputs):
    batch = np.asarray(inputs["batch"], dtype=np.float32)
    W = np.asarray(inputs["W"], dtype=np.float32)
    b_bias = np.asarray(inputs["b_bias"], dtype=np.float32)
    tt = np.asarray(inputs["token_types"]).astype(np.int64)
    pad = np.asarray(inputs["key_padding_mask"]).astype(bool)

    valid = ~pad                                   # [B, T]
    onehot = tt[:, None] == np.arange(G)[None, :]  # [T, G]
    counts = valid.astype(np.float32) @ onehot.astype(np.float32)  # [B, G]
    invc = np.where(counts > 0, 1.0 / np.maximum(counts, 1.0), 0.0).astype(
        np.float32
    ) / BSCALE

    core_tok = valid.reshape(NCORES, BL * T).sum(axis=1)
    ncp = int(max(core_tok + P - 1) // P)

    # w16[p, (g*DCH + c)*OUT + o] = W[g, c*128 + p, o]
    w16 = np.ascontiguousarray(
        W.reshape(G, DCH, P, OUT).transpose(2, 0, 1, 3)
    ).reshape(P, G * DCH * OUT).astype(NPF16)
    biasr = np.ascontiguousarray(
        np.broadcast_to(b_bias.reshape(1, G * OUT), (BL, G * OUT))
    )
    ident = np.eye(BG, dtype=np.float32)

    in_maps = []
    for cidx in range(NCORES):
        bs = slice(BL * cidx, BL * (cidx + 1))
        vb = valid[bs]                      # [BL, T]
        ib, it = np.nonzero(vb)             # b-major, t ascending
        n = len(ib)

        pk = np.zeros((ncp * P, D), dtype=NPE3)
        pk[:n] = (batch[bs][ib, it] * BSCALE).astype(NPE3)
        bt_dram = np.ascontiguousarray(
            pk.reshape(ncp, P, D).transpose(1, 0, 2)
        ).reshape(P, ncp * D)

        vf = np.zeros((ncp * P, BG), dtype=NPE3)
        g_of = tt[it]
        vf[np.arange(n), ib * G + g_of] = np.float32(1.0)
        vf_dram = np.ascontiguousarray(
            vf.reshape(ncp, P, BG).transpose(1, 0, 2)
        ).reshape(P, ncp * BG)

        in_maps.append(
            {
                "batch_pk": bt_dram,
                "vf": vf_dram,
                "w16": w16,
                "biasr": biasr,
                "invc": np.ascontiguousarray(
                    invc[bs].reshape(BG, 1)
                ),
                "ident": ident,
            }
        )
    return ncp, in_maps


def _gather(results):
    outs = [np.asarray(r["out"]).reshape(BL, G, OUT) for r in results]
    return np.ascontiguousarray(np.concatenate(outs, axis=0))


def kernel(**inputs) -> np.ndarray:
    ncp, in_maps = _prep(inputs)
    key = ("nc", ncp)
    if key not in _cache:
        _cache[key] = _build(ncp)
    res = bass_utils.run_bass_kernel_spmd(
        _cache[key], in_maps, core_ids=list(range(NCORES))
    )
    return _gather(res.results)


# revision 20
# speedup vs baseline: 1.1412x; 1.0234x over previous
"""Trainium2 Bass kernel for MeanTokenProjectionPool.

Computes, for batch [B,T,D], per-type segmented masked mean over T into G
groups followed by a per-group linear projection (W[g] @ mean + b[g]).

Strategy (data-parallel over B, 4 batch items per core, no cross-core comm):
  - ~50% of tokens are padding (key_padding_mask) and contribute nothing.
    The host packs only the VALID tokens of each core's 4 batch items into
    a contiguous stream laid out exactly as SBUF wants it
    [128 part, chunk, 512], so the batch DMA is a handful of large fully
    contiguous transfers at near-peak HBM bandwidth.
  - rel-err budget is 2e-2; the batch streams as fp8 e3m4 (1 B/elem,
    pre-scaled by 2 to use the dynamic range; measured end-to-end rel err
    1.27e-2 on the real data). The 0/1 membership mask vf is e3m4 too
    (exact). W keeps fp16 (its values sit in e3m4's subnormal range, and
    the kernel is PE-column-bound, not byte-bound, by this point).
  - One PE matmul per 128-token chunk accumulates all 32 (b,g) sums into a
    single PSUM bank; a per-partition DVE multiply by invc[b,g]/2 then
    yields means[32, 512] in f32. Batch tiles ramp [8,16,...] chunks so
    the first matmul fires early, and warm-up junk matmuls hold the PE
    clock at 2.4 GHz through the DMA lead-in.
  - Phase 2: PE-transpose means into [128d, 4c x 32bg], cast fp16, then per
    group g: out_g[4b, 512o] += mT[:,c,{b*8+g}] @ W16[g,c] over 4 d-chunks,
    one PSUM bank per group. W streams as 8 per-group fp16 tiles after the
    batch on the same HWDGE FIFO, so each group's GEMM fires as its W
    slice lands; a per-group bias add moves PSUM->SBUF.
  - Output per core is [4, G*OUT] f32; host reshapes/concats over cores.
"""

import ml_dtypes
import numpy as np

import concourse.bacc as bacc
import concourse.mybir as mybir
from concourse import bass_utils

from concourse.tile import TileContext

B, T, D, G, OUT = 32, 4096, 512, 8, 512
NCORES = 8
BL = B // NCORES   # batch items per core (4)
P = 128
DCH = D // P       # contraction chunks for the projection (4)
BG = BL * G        # combined (b,g) segments per core (32)
BSCALE = 2.0       # batch pre-scale into e3m4 range; undone via invc
NWARM = 48         # junk matmuls that hold the PE clock up during DMA lead-in

F32 = mybir.dt.float32
F16 = mybir.dt.float16
BF16 = mybir.dt.bfloat16
F8E3 = mybir.dt.float8e3
NPF16 = np.float16
NPE3 = ml_dtypes.float8_e3m4

_cache: dict = {}


def _tile_sizes(ncp: int):
    """Tile sizes in chunks: one small lead tile so phase 1 starts early,
    16-chunk steady state, small remainder tiles last so the batch tail's
    completion semaphore does not strand the final matmuls."""
    sizes = []
    for s in (8,):
        if sum(sizes) + s >= ncp:
            break
        sizes.append(s)
    while ncp - sum(sizes) > 16 + 8:
        sizes.append(16)
    rem = ncp - sum(sizes)
    sizes.append(rem - rem // 2)
    sizes.append(rem // 2)
    return [s for s in sizes if s > 0]


def _build(ncp: int):
    """Compile the SPMD program for a packed capacity of ncp 128-token
    chunks per core."""
    nc = bacc.Bacc(
        "TRN2", target_bir_lowering=False, debug=False, num_devices=NCORES
    )

    sizes = _tile_sizes(ncp)

    bt_d = nc.dram_tensor("batch_pk", [P, ncp * D], F8E3, kind="ExternalInput")
    vf_d = nc.dram_tensor("vf", [P, ncp * BG], F8E3, kind="ExternalInput")
    w_d = nc.dram_tensor("w16", [P, G * DCH * OUT], F16, kind="ExternalInput")
    bias_d = nc.dram_tensor("biasr", [BL, G * OUT], F32, kind="ExternalInput")
    invc_d = nc.dram_tensor("invc", [BG, 1], F32, kind="ExternalInput")
    ident_d = nc.dram_tensor("ident", [BG, BG], F32, kind="ExternalInput")
    out_d = nc.dram_tensor("out", [BL, G * OUT], F32, kind="ExternalOutput")

    with TileContext(nc) as tc:
        with tc.tile_pool(name="consts", bufs=1) as consts, \
             tc.tile_pool(name="bpool", bufs=len(sizes)) as bpool, \
             tc.tile_pool(name="wpool", bufs=G) as wpool:
            pacc_ctx = tc.tile_pool(name="pacc", bufs=1, space="PSUM")
            pacc = pacc_ctx.__enter__()
            ptp_ctx = tc.tile_pool(name="ptp", bufs=1, space="PSUM")
            ptp = ptp_ctx.__enter__()
            pjunk_ctx = tc.tile_pool(name="pjunk", bufs=1, space="PSUM")
            pjunk = pjunk_ctx.__enter__()

            # Entire input stream issues up-front on the sync HWDGE FIFO in
            # priority order; every tile has its own buffer so the ring
            # never stalls on buffer reuse. Small consts ride the scalar
            # HWDGE ring so they don't delay the batch stream. vf splits
            # into a head (covering the lead tiles) and rest so the first
            # matmuls only wait on a tiny mask DMA.
            vfsplit = min(sum(sizes[:2]), ncp)
            vfh_sb = consts.tile([P, vfsplit * BG], F8E3)
            vfr_sb = consts.tile([P, (ncp - vfsplit) * BG], F8E3)

            def vf_col(c):
                if c < vfsplit:
                    return vfh_sb[:, c * BG:(c + 1) * BG]
                cc = c - vfsplit
                return vfr_sb[:, cc * BG:(cc + 1) * BG]

            btiles = []
            c0 = 0
            for i, s in enumerate(sizes):
                bt = bpool.tile([P, s * D], F8E3, tag="bt")
                btiles.append((c0, c0 + s, bt))
                c0 += s

            def issue_bt(i):
                c0, c1, bt = btiles[i]
                nc.sync.dma_start(
                    out=bt, in_=bt_d.ap()[:, c0 * D:c1 * D]
                )

            issue_bt(0)
            nc.sync.dma_start(out=vfh_sb, in_=vf_d.ap()[:, :vfsplit * BG])
            if len(sizes) > 1:
                issue_bt(1)
            nc.sync.dma_start(out=vfr_sb, in_=vf_d.ap()[:, vfsplit * BG:])
            for i in range(2, len(sizes)):
                issue_bt(i)

            wtiles = []
            for g in range(G):
                wg = wpool.tile([P, DCH * OUT], F16, tag="wg")
                nc.sync.dma_start(
                    out=wg, in_=w_d.ap()[:, g * DCH * OUT:(g + 1) * DCH * OUT]
                )
                wtiles.append(wg)

            bias_sb = consts.tile([BL, G * OUT], F32)
            nc.scalar.dma_start(out=bias_sb, in_=bias_d.ap())
            invc_sb = consts.tile([BG, 1], F32)
            nc.scalar.dma_start(out=invc_sb, in_=invc_d.ap())
            ident = consts.tile([BG, BG], F32)
            nc.scalar.dma_start(out=ident, in_=ident_d.ap())

            means_sb = consts.tile([BG, D], F32)
            mt_sb = consts.tile([P, DCH, BG], F16)
            out_sb = consts.tile([BL, G * OUT], F32)

            # Junk matmuls: no data deps, so they run while the first DMAs
            # stream in, pushing the PE through the HAM half-clock window.
            junk_sb = consts.tile([P, P], BF16)
            nc.gpsimd.memset(junk_sb, 0.0)
            junk_ps = pjunk.tile([G, P], F32, tag="junk")
            for _ in range(NWARM):
                nc.tensor.matmul(
                    junk_ps, lhsT=junk_sb[:, :G], rhs=junk_sb,
                    start=True, stop=True,
                )

            # Phase 1: sums[32 (b,g), 512] accumulate in one PSUM bank.
            means_ps = pacc.tile([BG, D], F32, tag="means")
            for c0, c1, bt in btiles:
                for c in range(c0, c1):
                    nc.tensor.matmul(
                        means_ps,
                        lhsT=vf_col(c),
                        rhs=bt[:, (c - c0) * D:(c - c0 + 1) * D],
                        start=(c == 0), stop=(c == ncp - 1),
                    )
            # means = sums * (invc / BSCALE), per-(b,g)-partition scalar
            nc.vector.tensor_scalar_mul(means_sb, means_ps, invc_sb)

            # Transpose means -> mt [128 d, c, (b,g)] and cast to fp16.
            tp_ps = ptp.tile([P, DCH, BG], F32, tag="tp")
            for c in range(DCH):
                nc.tensor.transpose(
                    tp_ps[:, c, :], means_sb[:, c * P:(c + 1) * P], ident
                )
            nc.vector.tensor_copy(out=mt_sb, in_=tp_ps)

            # Phase 2: per-group projection, one PSUM bank per group.
            pjunk_ctx.__exit__(None, None, None)
            ptp_ctx.__exit__(None, None, None)
            pacc_ctx.__exit__(None, None, None)
            mt_v = mt_sb.rearrange("p c (b g) -> p c g b", g=G)
            with tc.tile_pool(name="pout", bufs=G, space="PSUM") as pout:
                for g in range(G):
                    og = pout.tile([BL, OUT], F32, tag="og", name=f"og{g}")
                    for c in range(DCH):
                        nc.tensor.matmul(
                            og,
                            lhsT=mt_v[:, c, g, :],
                            rhs=wtiles[g][:, c * OUT:(c + 1) * OUT],
                            start=(c == 0), stop=(c == DCH - 1),
                        )
                    # bias add + PSUM->SBUF copyback in one op
                    nc.vector.tensor_add(
                        out_sb[:, g * OUT:(g + 1) * OUT],
                        og,
                        bias_sb[:, g * OUT:(g + 1) * OUT],
                    )

            nc.sync.dma_start(out=out_d.ap(), in_=out_sb)

    nc.compile()
    return nc


def _prep(inputs):
    batch = np.asarray(inputs["batch"], dtype=np.float32)
    W = np.asarray(inputs["W"], dtype=np.float32)
    b_bias = np.asarray(inputs["b_bias"], dtype=np.float32)
    tt = np.asarray(inputs["token_types"]).astype(np.int64)
    pad = np.asarray(inputs["key_padding_mask"]).astype(bool)

    valid = ~pad                                   # [B, T]
    onehot = tt[:, None] == np.arange(G)[None, :]  # [T, G]
    counts = valid.astype(np.float32) @ onehot.astype(np.float32)  # [B, G]
    invc = np.where(counts > 0, 1.0 / np.maximum(counts, 1.0), 0.0).astype(
        np.float32
    ) / BSCALE

    core_tok = valid.reshape(NCORES, BL * T).sum(axis=1)
    ncp = int(max(core_tok + P - 1) // P)

    # w16[p, (g*DCH + c)*OUT + o] = W[g, c*128 + p, o]
    w16 = np.ascontiguousarray(
        W.reshape(G, DCH, P, OUT).transpose(2, 0, 1, 3)
    ).reshape(P, G * DCH * OUT).astype(NPF16)
    biasr = np.ascontiguousarray(
        np.broadcast_to(b_bias.reshape(1, G * OUT), (BL, G * OUT))
    )
    ident = np.eye(BG, dtype=np.float32)

    in_maps = []
    for cidx in range(NCORES):
        bs = slice(BL * cidx, BL * (cidx + 1))
        vb = valid[bs]                      # [BL, T]
        ib, it = np.nonzero(vb)             # b-major, t ascending
        n = len(ib)

        pk = np.zeros((ncp * P, D), dtype=NPE3)
        pk[:n] = (batch[bs][ib, it] * BSCALE).astype(NPE3)
        bt_dram = np.ascontiguousarray(
            pk.reshape(ncp, P, D).transpose(1, 0, 2)
        ).reshape(P, ncp * D)

        vf = np.zeros((ncp * P, BG), dtype=NPE3)
        g_of = tt[it]
        vf[np.arange(n), ib * G + g_of] = np.float32(1.0)
        vf_dram = np.ascontiguousarray(
            vf.reshape(ncp, P, BG).transpose(1, 0, 2)
        ).reshape(P, ncp * BG)

        in_maps.append(
            {
                "batch_pk": bt_dram,
                "vf": vf_dram,
                "w16": w16,
                "biasr": biasr,
                "invc": np.ascontiguousarray(
                    invc[bs].reshape(BG, 1)
                ),
                "ident": ident,
            }
        )
    return ncp, in_maps


def _gather(results):
    outs = [np.asarray(r["out"]).reshape(BL, G, OUT) for r in results]
    return np.ascontiguousarray(np.concatenate(outs, axis=0))


def kernel(**inputs) -> np.ndarray:
    ncp, in_maps = _prep(inputs)
    key = ("nc", ncp)
    if key not in _cache:
        _cache[key] = _build(ncp)
    res = bass_utils.run_bass_kernel_spmd(
        _cache[key], in_maps, core_ids=list(range(NCORES))
    )
    return _gather(res.results)
